# revision 71
# baseline (speedup 1.0000x reference)
"""DARNN (dual-attention RNN) Trainium2 Bass kernel.

Sharding: pure data-parallel over batch. B=256 -> 8 cores x 32.
All weights replicated; each core runs the full T-step recurrence on its
batch shard.

Key design points (per core, Bl=32 local batch):
- Both recurrences run as TWO independent half-batch chains (16 rows
  each).  The per-step dependency cycle is latency-bound (~20 links of
  engine-fixed cost + sem hops), so ops are kept few and large; the two
  chains overlap on complementary engines.
- PSUM accumulation only survives within one unbroken per-slice matmul
  group: all matmuls of a gate slice (bias/Whh/Wih) issue contiguously.
  Re-opening a closed group, or interleaving open groups on one tile,
  silently drops the earlier contributions.
- Encoder attention layout "L5": partition p = b_lo*64+u (u: WU_e out
  dim, 64), free = (n, b_hi).  E[b,n] contraction over u runs on the PE
  via accumulating matmuls with v-masked stationary tiles -> E lands in
  (b-part, n-free) layout for a fused exp+sum softmax.
- Decoder attention: partition = m' (WU_d out dim, 2x128), free = (t, b).
  l = v_d . tanh(...) via column-masked PE matmuls into a (DCH, TL*HB)
  PSUM tile per chain; softmax over t runs in-layout: exp straight from
  PSUM, one strided X-axis tensor_reduce over t_loc, and one DCH->1
  ones-matmul produce den and num together.  No per-step DMA.
- softmax context is never materialized: the model output is linear in
  ctx, so ctx.w2 == sum_t beta q2 with q2 = Xe @ w2 precomputed once.
- sigmoid(x) = 0.5 + 0.5*tanh(x/2): keeps the single ACT table set
  (exp_and_others: exp + tanh) loaded -> no 2.7us table switches.
- LSTM gate bias enters as a K=1/K=2 matmul inside each gate slice's
  contiguous PSUM group; gate tanh reads PSUM directly; h is written
  directly in bf16; carried c stays fp32 only (the c-part recurrent
  matmuls use fp32 weights, so no bf16 shadow copy of c).
- bf16 for the big attention adds (DVE 2x mode) / tanh outputs / matmul
  operands; fp32 carried state and softmax.
"""

import sys

for _p in ("/opt/trn_rl_repo", "/root/.axon_site/_ro/trn_rl_repo"):
    if _p not in sys.path:
        sys.path.insert(0, _p)

import numpy as np

B, T, N, M, P, YD = 256, 64, 128, 256, 256, 1
NCORES = 8
BL = B // NCORES  # 32 local batch
HB = BL // 2  # 16 per chain
U = T  # encoder attention hidden dim (64)
DCH = 8  # decoder attention chunk count (t-chunks)
TL = T // DCH  # t per chunk


def _f32(x):
    return np.ascontiguousarray(x, dtype=np.float32)


def _prep_weights(inputs):
    """Host-side weight re-layout + folding (weights only; no input data)."""
    import ml_dtypes

    bf16 = ml_dtypes.bfloat16

    def _bf(x):
        return np.ascontiguousarray(np.asarray(x, np.float32).astype(bf16))

    WU_e = _f32(inputs["WU_e"])  # (64, 576)
    v_e = _f32(inputs["v_e"])  # (1, 64)
    WU_d = _f32(inputs["WU_d"])  # (256, 768)
    v_d = _f32(inputs["v_d"])  # (1, 256)
    wb = _f32(inputs["wb_tilde"])  # (1, 257)
    Wih_e = _f32(inputs["Wih_e"])  # (1024, 128)
    Whh_e = _f32(inputs["Whh_e"])  # (1024, 256)
    be = _f32(inputs["bih_e"]) + _f32(inputs["bhh_e"])  # (1024,)
    Wih_d = _f32(inputs["Wih_d"])  # (1024, 1)
    Whh_d = _f32(inputs["Whh_d"])  # (1024, 256)
    bd = _f32(inputs["bih_d"]) + _f32(inputs["bhh_d"])  # (1024,)
    Wb_W = _f32(inputs["Wb_W"])  # (256, 512)
    Wb_b = _f32(inputs["Wb_b"])  # (256,)
    vb_W = _f32(inputs["vb_W"])  # (1, 256)
    vb_b = _f32(inputs["vb_b"])  # (1,)

    Wh_e = WU_e[:, : 2 * M]  # (64, 512)
    Wx_e = WU_e[:, 2 * M :]  # (64, 64)
    Wh_d = WU_d[:, : 2 * P]  # (256, 512)
    Wx_d = WU_d[:, 2 * P :]  # (256, 256)

    w = {}

    # --- encoder attention ---
    # s-matmul lhsT per K-tile k: [r, k, u] = Wh_e[u, k*128+r]  (partition dim first)
    # h-part (k=0,1) stays bf16 (rhs is XeT); c-part (k=2,3) is fp32 so the
    # matmul can read the fp32 carried c state directly (no bf16 shadow copy)
    w["enc_wh"] = _bf(
        np.stack([Wh_e[:, k * 128 : (k + 1) * 128].T for k in range(2)], axis=1)
    )  # (128, 2, 64)
    w["enc_whc"] = _f32(
        np.stack([Wh_e[:, k * 128 : (k + 1) * 128].T for k in range(2, 4)], axis=1)
    )  # (128, 2, 64)
    # A5 lhsT: (64, 64), [t, u] = Wx_e[u, t]
    w["enc_wx"] = _f32(Wx_e.T)  # (64, 64)
    # E contraction masks, b = b_hi*2 + b_lo: [(b_lo*64+u), j=b_hi, b]
    vm = np.zeros((128, 16, 32), np.float32)
    for j in range(16):
        for b_lo in range(2):
            vm[b_lo * 64 : b_lo * 64 + 64, j, j * 2 + b_lo] = v_e[0]
    w["enc_vmask"] = _bf(vm)

    # --- encoder LSTM (gate order i,f,g,o; sigmoid-gates prescaled by 0.5) ---
    gate_scale = np.ones((4 * M,), np.float32)
    gate_scale[0 : 2 * M] = 0.5  # i, f
    gate_scale[3 * M :] = 0.5  # o
    Wih_s = Wih_e * gate_scale[:, None]
    Whh_s = Whh_e * gate_scale[:, None]
    be_s = be * gate_scale
    # wih lhsT per j: [n, j, c] = Wih_s[j*128+c, n]
    w["enc_wih"] = _bf(
        np.stack([Wih_s[j * 128 : (j + 1) * 128, :].T for j in range(8)], axis=1)
    )  # (128, 8, 128)
    w["enc_whh"] = _bf(
        np.stack(
            [
                np.stack(
                    [
                        Whh_s[j * 128 : (j + 1) * 128, k * 128 : (k + 1) * 128].T
                        for k in range(2)
                    ],
                    axis=1,
                )
                for j in range(8)
            ],
            axis=1,
        )
    )  # (128, 8, 2, 128)
    # bias as a K=1 lhsT row: (1, 8, 128)
    w["enc_biasb"] = _f32(be_s.reshape(1, 8, 128))

    # --- decoder attention ---
    # sd lhsT: (128, 2, 2, 128): [r, k, mt, c] = Wh_d[mt*128+c, k*128+r]
    # h-part (k=0,1) bf16; c-part (k=2,3) fp32 (reads fp32 cd state directly)
    w["dec_wh"] = _bf(
        np.stack(
            [
                np.stack(
                    [
                        Wh_d[mt * 128 : (mt + 1) * 128, k * 128 : (k + 1) * 128].T
                        for mt in range(2)
                    ],
                    axis=1,
                )
                for k in range(2)
            ],
            axis=1,
        )
    )
    w["dec_whc"] = _f32(
        np.stack(
            [
                np.stack(
                    [
                        Wh_d[mt * 128 : (mt + 1) * 128, k * 128 : (k + 1) * 128].T
                        for mt in range(2)
                    ],
                    axis=1,
                )
                for k in range(2, 4)
            ],
            axis=1,
        )
    )
    # AX lhsT: (128, 2, 2, 128): [r, k, mt, c] = Wx_d[mt*128+c, k*128+r]
    w["dec_wx"] = _bf(
        np.stack(
            [
                np.stack(
                    [
                        Wx_d[mt * 128 : (mt + 1) * 128, k * 128 : (k + 1) * 128].T
                        for mt in range(2)
                    ],
                    axis=1,
                )
                for k in range(2)
            ],
            axis=1,
        )
    )
    # l contraction masks: (128, 2, DCH, DCH): [r, k, c, col] = v_d[k*128+r]*(col==c)
    dvm = np.zeros((128, 2, DCH, DCH), np.float32)
    for k in range(2):
        for c in range(DCH):
            dvm[:, k, c, c] = v_d[0, k * 128 : (k + 1) * 128]
    w["dec_vmask"] = _bf(dvm)

    # output fold: y = [h, ctx] @ w_eff + c_eff
    w_eff = Wb_W.T @ vb_W.T  # (512, 1)
    c_eff = float((Wb_b @ vb_W.T + vb_b)[0])
    wbm = wb[0, 1:]  # (256,)
    w2 = w_eff[256:, 0]  # (256,)
    # q / q2 masks like dec_vmask but with wbm / w2
    qm = np.zeros((128, 2, DCH, DCH), np.float32)
    q2m = np.zeros((128, 2, DCH, DCH), np.float32)
    for k in range(2):
        for c in range(DCH):
            qm[:, k, c, c] = wbm[k * 128 : (k + 1) * 128]
            q2m[:, k, c, c] = w2[k * 128 : (k + 1) * 128]
    w["q_mask"] = _bf(qm)
    w["q2_mask"] = _bf(q2m)

    # --- decoder LSTM ---
    Wih_ds = Wih_d * gate_scale[:, None]
    Whh_ds = Whh_d * gate_scale[:, None]
    bd_s = bd * gate_scale
    # decoder input weight + bias as K=2 lhsT rows: (2, 8, 128)
    w["dec_wihb"] = _f32(
        np.stack([Wih_ds[:, 0].reshape(8, 128), bd_s.reshape(8, 128)], axis=0)
    )
    w["dec_whh"] = _bf(
        np.stack(
            [
                np.stack(
                    [
                        Whh_ds[j * 128 : (j + 1) * 128, k * 128 : (k + 1) * 128].T
                        for k in range(2)
                    ],
                    axis=1,
                )
                for j in range(8)
            ],
            axis=1,
        )
    )  # (128, 8, 2, 128)

    # final h projection lhsT: (128, 2, 1): [r, k, 0]
    w["w1f"] = _bf(
        np.stack([w_eff[k * 128 : (k + 1) * 128, :] for k in range(2)], axis=1)
    )

    w["ones4"] = _f32(np.ones((DCH, 1), np.float32))
    import ml_dtypes as _md

    w["i16bf"] = np.ascontiguousarray(np.eye(16, dtype=np.float32).astype(_md.bfloat16))

    scalars = {"wb0": float(wb[0, 0]), "c_eff": c_eff}
    return w, scalars


def _build(w_shapes, scalars):
    """Build the SPMD Bass program (same program for all 8 cores)."""
    import concourse.bass as bass
    import concourse.bacc as bacc
    import concourse.tile as tile
    from concourse import mybir

    fp32 = mybir.dt.float32
    bf16 = mybir.dt.bfloat16
    AF = mybir.ActivationFunctionType
    OP = mybir.AluOpType

    nc = bacc.Bacc()

    inp = nc.dram_tensor("inp", [BL, T, N + YD], fp32, kind="ExternalInput")
    dram = {
        name: nc.dram_tensor(name, list(shape), dt, kind="ExternalInput")
        for name, (shape, dt) in w_shapes.items()
    }
    out_d = nc.dram_tensor("out", [BL, YD], fp32, kind="ExternalOutput")

    wb0 = scalars["wb0"]
    c_eff = scalars["c_eff"]

    with tile.TileContext(nc) as tc:
        import contextlib

        ctx = contextlib.ExitStack()
        with ctx:
            sing = ctx.enter_context(tc.tile_pool(name="sing", bufs=1))

            # ---------- load constants / weights to SBUF ----------
            sb = {}
            for name, t_ in dram.items():
                shp = list(t_.shape)
                dt = t_.dtype
                til = sing.tile(shp, dt, name=f"w_{name}", tag=f"w_{name}")
                nc.gpsimd.dma_start(out=til, in_=t_.ap())
                sb[name] = til

            # input staging
            x_all = sing.tile([64, BL, 128], fp32, name="x_all", tag="x_all")
            nc.gpsimd.dma_start(
                out=x_all,
                in_=bass.AP(
                    tensor=inp.ap().tensor,
                    offset=0,
                    ap=[[129, 64], [64 * 129, BL], [1, 128]],
                ),
            )
            # halves side-by-side on partitions 0:16, half index in free dim
            x_bp = sing.tile([16, 2, T, 128], fp32, name="x_bp", tag="x_bp")
            for hh in range(2):
                nc.gpsimd.dma_start(
                    out=x_bp[:, hh, :, :],
                    in_=bass.AP(
                        tensor=inp.ap().tensor,
                        offset=hh * 16 * 64 * 129,
                        ap=[[64 * 129, 16], [129, T], [1, 128]],
                    ),
                )
            # Y on a single partition: y_row[0, t*BL + b] = Y[b, t]
            y_row = sing.tile([1, (T - 1) * BL], fp32, name="y_row", tag="y_row")
            nc.gpsimd.dma_start(
                out=y_row,
                in_=bass.AP(
                    tensor=inp.ap().tensor,
                    offset=128,
                    ap=[[129, T - 1], [64 * 129, BL]],
                ),
            )

            # DMA-wait absorbers: a DMA-semaphore wait costs 2 of an
            # instruction's ~3 sync slots, so real consumers can't afford
            # them alongside their data deps.  Touch each DMA'd tensor once
            # per engine with a dep-free op so each engine's vector clock
            # observes every DMA queue up front.
            tch_v = sing.tile([1, 8], fp32, name="tch_v", tag="tch_v")
            tch_a = sing.tile([1, 8], fp32, name="tch_a", tag="tch_a")
            touch_list = [x_all[0:1, 0:1, 0:1], x_bp[0:1, 0:1, 0:1, 0:1], y_row[0:1, 0:1]]
            for name in sorted(sb.keys()):
                sl = sb[name]
                while len(sl.shape) > 2:
                    sl = sl[:, 0]
                touch_list.append(sl[0:1, 0:1])
            for i, ap in enumerate(touch_list):
                nc.vector.tensor_copy(out=tch_v[0:1, i % 8 : i % 8 + 1], in_=ap)
                nc.scalar.copy(out=tch_a[0:1, i % 8 : i % 8 + 1], in_=ap)
                if ap.dtype == bf16:
                    nc.tensor.ldweights(ap)
                else:
                    nc.tensor.ldweights(ap.bitcast(bf16))

            # persistent state (per chain, so the dep tracker never sees the
            # two chains touch the same tile)
            A5 = sing.tile([128, 128, 16], bf16, name="A5", tag="A5")
            XeTh = [
                sing.tile([128, 2, T, HB], bf16, name=f"XeT{h}", tag=f"XeT{h}")
                for h in range(2)
            ]
            c_sth = [
                sing.tile([128, 2, HB], fp32, name=f"c_st{h}", tag=f"c_st{h}")
                for h in range(2)
            ]

            # ---------- global PSUM pools (8 banks total) ----------
            ps_att = ctx.enter_context(tc.tile_pool(name="ps_att", bufs=2, space="PSUM"))
            ps_s = ctx.enter_context(tc.tile_pool(name="ps_sx", bufs=2, space="PSUM"))
            ps_g = ctx.enter_context(tc.tile_pool(name="ps_gx", bufs=2, space="PSUM"))
            ps_misc = ctx.enter_context(tc.tile_pool(name="ps_misc", bufs=2, space="PSUM"))

            # ---------- encoder precompute: A5 ----------
            for j in range(16):
                a5p = ps_att.tile([128, 512], fp32, tag="att", name="a5p")[:, 0:128]
                for b_lo in range(2):
                    nc.tensor.matmul(
                        a5p[b_lo * 64 : b_lo * 64 + 64, :],
                        lhsT=sb["enc_wx"],
                        rhs=x_all[:, j * 2 + b_lo, :],
                        start=True,
                        stop=True,
                    )
                nc.vector.tensor_copy(out=A5[:, :, j], in_=a5p)

            # ---------- encoder loop (two offset half-batch chains) ----------
            enc_loop = ctx.enter_context(contextlib.ExitStack())
            sp = enc_loop.enter_context(tc.tile_pool(name="sp", bufs=2))
            sp3 = enc_loop.enter_context(tc.tile_pool(name="sp3", bufs=3))

            on16 = sing.tile([1, 16], fp32, name="on16", tag="on16")
            nc.vector.memset(on16, 1.0)

            XeT_vh = [
                XeTh[h].rearrange("p k t (bh bl) -> p k t bh bl", bl=2)
                for h in range(2)
            ]
            c_st_vh = [
                c_sth[h].rearrange("p k (bh bl) -> p k bh bl", bl=2) for h in range(2)
            ]

            est = [[{} for _ in range(T)], [{} for _ in range(T)]]

            def efront(h, t):
                d = est[h][t]
                bs = slice(h * 16, (h + 1) * 16)
                hs8 = slice(h * 8, (h + 1) * 8)
                d["bs"], d["hs8"] = bs, hs8
                # s = Wh_e . [h; c]  (8 tiny matmuls)
                if t > 0:
                    s_ps_t = ps_s.tile([128, 2, 32], fp32, tag="s", name="s_ps_t")
                    s_ps = s_ps_t[:, 0, 0:8]
                    for b_lo in range(2):
                        for k in range(4):
                            if k < 2:
                                lhsT = sb["enc_wh"][:, k, :]
                                rhs = XeT_vh[h][:, k, t - 1, :, b_lo]
                            else:
                                lhsT = sb["enc_whc"][:, k - 2, :]
                                rhs = c_st_vh[h][:, k - 2, :, b_lo]
                            nc.tensor.matmul(
                                s_ps[b_lo * 64 : b_lo * 64 + 64, :],
                                lhsT=lhsT,
                                rhs=rhs,
                                start=(k == 0),
                                stop=(k == 3),
                            )
                # tanh input and tanh, in two 4-column halves so the first
                # half's E matmuls overlap the second half's tanh
                if t > 0:
                    s_bf = sp.tile([128, 8], bf16, tag=f"s_bf{h}")
                    nc.vector.tensor_copy(out=s_bf, in_=s_ps)
                th5 = sp.tile([128, 128, 8], bf16, tag=f"th5{h}")
                for gg in range(2):
                    jsl = slice(gg * 4, gg * 4 + 4)
                    hsl = slice(h * 8 + gg * 4, h * 8 + gg * 4 + 4)
                    if t > 0:
                        th_in = sp.tile([128, 128, 4], bf16, tag=f"th_in{h}{gg}")
                        s_bc = s_bf[:, jsl][:, None, :].broadcast_to([128, 128, 4])
                        nc.vector.tensor_add(th_in, A5[:, :, hsl], s_bc)
                        src = th_in
                    else:
                        src = A5[:, :, hsl]
                    nc.scalar.activation(out=th5[:, :, jsl], in_=src, func=AF.Tanh)
                d["th5"] = th5
                # E contraction
                e_ps = ps_att.tile([16, 512], fp32, tag="att", name="e_ps")[:, 0:128]
                for jj in range(8):
                    nc.tensor.matmul(
                        e_ps,
                        lhsT=sb["enc_vmask"][:, h * 8 + jj, bs],
                        rhs=th5[:, :, jj],
                        start=(jj == 0),
                        stop=(jj == 7),
                    )
                d["e_ps"] = e_ps

            def eback(h, t):
                d = est[h][t]
                bs, hs8 = d["bs"], d["hs8"]
                # softmax over n + alpha*x, transposed to (n, b)
                expE = sp.tile([16, 128], fp32, tag=f"expE{h}")
                zsum = sp.tile([16, 1], fp32, tag=f"zsum{h}")
                nc.scalar.activation(out=expE, in_=d["e_ps"], func=AF.Exp, accum_out=zsum)
                invz = sp.tile([16, 1], fp32, tag=f"invz{h}")
                nc.vector.reciprocal(out=invz, in_=zsum)
                xe = sp.tile([16, 128], fp32, tag=f"xe{h}")
                nc.vector.tensor_mul(xe, expE, x_bp[:, h, t, :])
                xa = sp.tile([16, 128], bf16, tag=f"xa{h}")
                nc.vector.tensor_scalar(
                    out=xa, in0=xe, scalar1=invz, scalar2=None, op0=OP.mult
                )
                xt_ps = ps_misc.tile([128, 32], bf16, tag="misc", name="xt_ps")[:, 0:16]
                nc.tensor.matmul(xt_ps, lhsT=xa, rhs=sb["i16bf"], is_transpose=True)
                xaT = sp.tile([128, 16], bf16, tag=f"xaT{h}")
                nc.vector.tensor_copy(out=xaT, in_=xt_ps)
                d["xaT"] = xaT
                # gates: per-j matmul groups must be contiguous — PSUM
                # accumulation only survives within one unbroken group
                g_ps = ps_g.tile([128, 8, 16], fp32, tag="g", name="g_ps_t")
                d["g_ps"] = g_ps
                for j in range(8):
                    nc.tensor.matmul(
                        g_ps[:, j, :],
                        lhsT=sb["enc_biasb"][:, j, :],
                        rhs=on16,
                        start=True,
                        stop=False,
                    )
                    if t > 0:
                        for k in range(2):
                            nc.tensor.matmul(
                                g_ps[:, j, :],
                                lhsT=sb["enc_whh"][:, j, k, :],
                                rhs=XeTh[h][:, k, t - 1, :],
                                start=False,
                                stop=False,
                            )
                    nc.tensor.matmul(
                        g_ps[:, j, :],
                        lhsT=sb["enc_wih"][:, j, :],
                        rhs=xaT,
                        start=False,
                        stop=True,
                    )
                # LSTM cell
                tg = sp.tile([128, 8, 16], fp32, tag=f"tg{h}")
                nc.scalar.activation(
                    out=tg.rearrange("p j b -> p (j b)"),
                    in_=d["g_ps"].rearrange("p j b -> p (j b)"),
                    func=AF.Tanh,
                )
                # one tensor_scalar over all 8 gate rows; rows 4:6 (g) come
                # out wrong but are never read from sig
                sig = sp.tile([128, 8, 16], fp32, tag=f"sig{h}")
                nc.vector.tensor_scalar(
                    out=sig, in0=tg,
                    scalar1=0.5, scalar2=0.5, op0=OP.mult, op1=OP.add,
                )
                tmp2 = sp3.tile([128, 2, 16], fp32, tag=f"tmp2{h}")
                nc.vector.tensor_mul(tmp2, sig[:, 0:2, :], tg[:, 4:6, :])
                if t > 0:
                    tmp1 = sp3.tile([128, 2, 16], fp32, tag=f"tmp1{h}")
                    nc.vector.tensor_mul(tmp1, sig[:, 2:4, :], c_sth[h])
                    nc.vector.tensor_add(c_sth[h], tmp1, tmp2)
                else:
                    nc.vector.tensor_copy(out=c_sth[h], in_=tmp2)
                tcn = sp3.tile([128, 2, 16], fp32, tag=f"tcn{h}")
                nc.scalar.activation(out=tcn, in_=c_sth[h], func=AF.Tanh)
                nc.vector.tensor_mul(XeTh[h][:, :, t, :], sig[:, 6:8, :], tcn)

            # whole-step block alternation: each chain's step is issued as
            # one block, so the other chain's (independent) block fills each
            # engine's queue while this chain waits on its recurrence.  The
            # chains have no cross deps, so without intervention they settle
            # in phase and fight for the same engine at the same time; a
            # one-time fence (each engine touches chain 0's mid-step xaT
            # before any chain-1 work) starts chain 1 half a cycle late and
            # the offset persists.
            # two-stage software pipeline: windows alternate [f1(t), b0(t)] /
            # [f0(t+1), b1(t)] so each chain's front and back land in
            # different windows (step period ~ cycle/2).  The chains are
            # dep-free at t=0 and would start — and stay — in lockstep, so a
            # one-time fence delays chain 1's first front by ~half a cycle
            # (every engine touches chain 0's mid-step xaT first).
            for t in range(T):
                efront(0, t)
                eback(0, t)
                efront(1, t)
                eback(1, t)

            enc_loop.close()

            # ---------- decoder precompute: AX, qs, qs2 (all per chain) ----------
            AXh = [
                [
                    sing.tile([128, T, HB], bf16, name=f"AX{mt}_{h}", tag=f"AX{mt}_{h}")
                    for mt in range(2)
                ]
                for h in range(2)
            ]
            # qs[h]: (DCH, 2, TL*HB): [ch, which(q|q2), t_loc*16+b]
            qs = [
                sing.tile([DCH, 2, TL * HB], fp32, name=f"qs{h}", tag=f"qs{h}")
                for h in range(2)
            ]
            for h in range(2):
                for mt in range(2):
                    for chq in range(4):
                        axp = ps_att.tile([128, 512], fp32, tag="att", name="axp")[
                            :, 0:256
                        ]
                        for k in range(2):
                            nc.tensor.matmul(
                                axp,
                                lhsT=sb["dec_wx"][:, k, mt, :],
                                rhs=XeTh[h][
                                    :, k, chq * 16 : (chq + 1) * 16, :
                                ].rearrange("p t b -> p (t b)"),
                                start=(k == 0),
                                stop=(k == 1),
                            )
                        nc.scalar.copy(
                            out=AXh[h][mt][
                                :, chq * 16 : (chq + 1) * 16, :
                            ].rearrange("p t b -> p (t b)"),
                            in_=axp,
                        )
                for which, mask in ((0, "q_mask"), (1, "q2_mask")):
                    qp = ps_misc.tile([128, 256], fp32, tag="misc", name="qp")[
                        0:DCH, 0 : TL * HB
                    ]
                    first = True
                    for ch in range(DCH):
                        for k in range(2):
                            nc.tensor.matmul(
                                qp,
                                lhsT=sb[mask][:, k, ch, :],
                                rhs=XeTh[h][
                                    :, k, ch * TL : (ch + 1) * TL, :
                                ].rearrange("p t b -> p (t b)"),
                                start=first,
                                stop=(ch == DCH - 1 and k == 1),
                            )
                            first = False
                    nc.vector.tensor_copy(out=qs[h][:, which, :], in_=qp)

            # ---------- decoder state (per chain) ----------
            hdh = [
                sing.tile([128, 2, HB], bf16, name=f"hd{h}", tag=f"hd{h}")
                for h in range(2)
            ]
            cdh = [
                sing.tile([128, 2, HB], fp32, name=f"cd{h}", tag=f"cd{h}")
                for h in range(2)
            ]
            yth = [
                sing.tile([2, HB], fp32, name=f"yt{h}", tag=f"yt{h}") for h in range(2)
            ]
            invh = [
                sing.tile([1, HB], fp32, name=f"inv{h}", tag=f"inv{h}")
                for h in range(2)
            ]
            for h in range(2):
                # row 1 must be ones (bias lane); row 0 is rewritten per step
                nc.vector.memset(yth[h], 1.0)
            # e2[h][:, 0, :] = exp(l), [:, 1, :] = exp(l)*q  (persistent; the
            # last step's exp feeds the q2 fold after the loop)
            e2 = [
                sing.tile([DCH, 2, TL * HB], fp32, name=f"e2_{h}", tag=f"e2_{h}")
                for h in range(2)
            ]
            ysc_all = sing.tile([1, (T - 1) * BL], fp32, name="ysc_all", tag="ysc_all")
            nc.vector.tensor_scalar(
                out=ysc_all, in0=y_row, scalar1=wb0, scalar2=None, op0=OP.mult
            )
            yfin = sing.tile([1, BL], fp32, name="yfin", tag="yfin")

            dec_loop = ctx.enter_context(contextlib.ExitStack())
            dp = dec_loop.enter_context(tc.tile_pool(name="dp", bufs=2))
            dp3 = dec_loop.enter_context(tc.tile_pool(name="dp3", bufs=3))

            dst = [[{} for _ in range(T - 1)], [{} for _ in range(T - 1)]]

            def dfront(h, t):
                d = dst[h][t]
                bs = slice(h * HB, (h + 1) * HB)
                d["bs"] = bs
                # recurrent-input matmuls (hd/cd from step t-1)
                if t > 0:
                    sd_ps_t = ps_s.tile([128, 2, 32], fp32, tag="s", name="sd_ps_t")
                    sd_ps = sd_ps_t[:, :, 0:HB]
                    for mt in range(2):
                        for k in range(4):
                            if k < 2:
                                lhsT = sb["dec_wh"][:, k, mt, :]
                                rhs = hdh[h][:, k, :]
                            else:
                                lhsT = sb["dec_whc"][:, k - 2, mt, :]
                                rhs = cdh[h][:, k - 2, :]
                            nc.tensor.matmul(
                                sd_ps[:, mt, :],
                                lhsT=lhsT,
                                rhs=rhs,
                                start=(k == 0),
                                stop=(k == 3),
                            )
                if t > 0:
                    sd_bf = dp.tile([128, 2, HB], bf16, tag=f"sd_bf{h}")
                    nc.vector.tensor_copy(out=sd_bf, in_=sd_ps)
                # attention tanh (two m'-tiles), each in two t-halves so the
                # first half's l matmuls overlap the second half's tanh
                th_d = []
                for mt in range(2):
                    thd = dp.tile([128, T, HB], bf16, tag=f"thd{mt}{h}")
                    for gg in range(2):
                        tsl = slice(gg * (T // 2), (gg + 1) * (T // 2))
                        if t > 0:
                            thi = dp.tile(
                                [128, T // 2, HB], bf16, tag=f"thi{mt}{h}{gg}"
                            )
                            sd_bc = sd_bf[:, mt, None, :].broadcast_to(
                                [128, T // 2, HB]
                            )
                            nc.vector.tensor_add(thi, AXh[h][mt][:, tsl, :], sd_bc)
                            src = thi
                        else:
                            src = AXh[h][mt][:, tsl, :]
                        nc.scalar.activation(
                            out=thd[:, tsl, :].rearrange("p t b -> p (t b)"),
                            in_=src,
                            func=AF.Tanh,
                        )
                    th_d.append(thd)
                # l = v_d . th -> (DCH, TL*HB) psum, layout (ch, t_loc, b)
                l_ps = ps_att.tile([32, 512], fp32, tag="att", name="l_ps")[
                    0:DCH, 0 : TL * HB
                ]
                first = True
                for k in range(2):  # k-major: mt0 contraction overlaps mt1 tanh
                    for ch in range(DCH):
                        nc.tensor.matmul(
                            l_ps,
                            lhsT=sb["dec_vmask"][:, k, ch, :],
                            rhs=th_d[k][:, ch * TL : (ch + 1) * TL, :].rearrange(
                                "p t b -> p (t b)"
                            ),
                            start=first,
                            stop=(k == 1 and ch == DCH - 1),
                        )
                        first = False
                d["l_ps"] = l_ps

            def dback(h, t):
                d = dst[h][t]
                bs = d["bs"]
                # softmax over t, in-layout: exp from PSUM, one strided X-axis
                # reduce over t_loc, one DCH->1 matmul for den|num together
                nc.scalar.activation(out=e2[h][:, 0, :], in_=d["l_ps"], func=AF.Exp)
                nc.vector.tensor_mul(e2[h][:, 1, :], e2[h][:, 0, :], qs[h][:, 0, :])
                dnm = dp.tile([DCH, 2, HB], fp32, tag=f"dnm{h}")
                nc.vector.tensor_reduce(
                    out=dnm,
                    in_=e2[h].rearrange("p k (tl b) -> p k b tl", b=HB),
                    axis=mybir.AxisListType.X,
                    op=OP.add,
                )
                dn_ps = ps_misc.tile([128, 64], fp32, tag="misc", name="dn_ps")[
                    0:1, 0:32
                ]
                nc.tensor.matmul(
                    dn_ps,
                    lhsT=sb["ones4"],
                    rhs=dnm.rearrange("p k b -> p (k b)"),
                    start=True,
                    stop=True,
                )
                nc.vector.reciprocal(out=invh[h], in_=dn_ps[:, 0:HB])
                nd = dp.tile([1, HB], fp32, tag=f"nd{h}")
                nc.vector.tensor_mul(nd, dn_ps[:, HB : 2 * HB], invh[h])
                nc.vector.tensor_add(
                    yth[h][0:1, :],
                    ysc_all[0:1, t * BL + h * HB : t * BL + (h + 1) * HB],
                    nd,
                )
                # LSTM gates: per-j contiguous groups; recurrent-weight
                # matmuls (ready at step start) lead each group so the 4-deep
                # wait queue hoists them past the stall on yt; the late
                # K=2 y~/bias matmul closes the group
                g_ps = ps_g.tile([128, 8, 16], fp32, tag="g", name="g_ps_d")
                for j in range(8):
                    if t > 0:
                        for k in range(2):
                            nc.tensor.matmul(
                                g_ps[:, j, :],
                                lhsT=sb["dec_whh"][:, j, k, :],
                                rhs=hdh[h][:, k, :],
                                start=(k == 0),
                                stop=False,
                            )
                    nc.tensor.matmul(
                        g_ps[:, j, :],
                        lhsT=sb["dec_wihb"][:, j, :],
                        rhs=yth[h],
                        start=(t == 0),
                        stop=True,
                    )
                tg = dp.tile([128, 8, HB], fp32, tag=f"tg{h}")
                nc.scalar.activation(
                    out=tg.rearrange("p j b -> p (j b)"),
                    in_=g_ps.rearrange("p j b -> p (j b)"),
                    func=AF.Tanh,
                )
                # one tensor_scalar over all 8 gate rows; rows 4:6 (g) come
                # out wrong but are never read from sig
                sig = dp.tile([128, 8, HB], fp32, tag=f"sig{h}")
                nc.vector.tensor_scalar(
                    out=sig, in0=tg,
                    scalar1=0.5, scalar2=0.5, op0=OP.mult, op1=OP.add,
                )
                tmp2 = dp3.tile([128, 2, HB], fp32, tag=f"tmp2{h}")
                nc.vector.tensor_mul(tmp2, sig[:, 0:2, :], tg[:, 4:6, :])
                if t > 0:
                    tmp1 = dp3.tile([128, 2, HB], fp32, tag=f"tmp1{h}")
                    nc.vector.tensor_mul(tmp1, sig[:, 2:4, :], cdh[h])
                    nc.vector.tensor_add(cdh[h], tmp1, tmp2)
                else:
                    nc.vector.tensor_copy(out=cdh[h], in_=tmp2)
                tcn = dp3.tile([128, 2, HB], fp32, tag=f"tcn{h}")
                nc.scalar.activation(
                    out=tcn.rearrange("p k b -> p (k b)"), in_=cdh[h], func=AF.Tanh
                )
                nc.vector.tensor_mul(hdh[h], sig[:, 6:8, :], tcn)

            def dfinal(h):
                bs = slice(h * HB, (h + 1) * HB)
                f0 = dp.tile([DCH, TL * HB], fp32, tag=f"f0{h}")
                nc.vector.tensor_mul(f0, e2[h][:, 0, :], qs[h][:, 1, :])
                f1 = dp.tile([DCH, HB], fp32, tag=f"f1{h}")
                nc.vector.tensor_reduce(
                    out=f1,
                    in_=f0.rearrange("p (tl b) -> p b tl", b=HB),
                    axis=mybir.AxisListType.X,
                    op=OP.add,
                )
                fin_ps = ps_misc.tile([128, 64], fp32, tag="misc", name="fin_ps")[
                    0:1, :
                ]
                nc.tensor.matmul(
                    fin_ps[:, 0:HB], lhsT=sb["ones4"], rhs=f1, start=True, stop=True
                )
                for k in range(2):
                    nc.tensor.matmul(
                        fin_ps[:, 32 : 32 + HB],
                        lhsT=sb["w1f"][:, k, :],
                        rhs=hdh[h][:, k, :],
                        start=(k == 0),
                        stop=(k == 1),
                    )
                nd2 = dp.tile([1, HB], fp32, tag=f"nd{h}")
                nc.vector.tensor_mul(nd2, fin_ps[:, 0:HB], invh[h])
                yf = dp.tile([1, HB], fp32, tag=f"yf{h}")
                nc.vector.tensor_add(yf, nd2, fin_ps[:, 32 : 32 + HB])
                nc.vector.tensor_scalar_add(yfin[:, bs], yf, c_eff)

            for t in range(T - 1):
                dfront(0, t)
                dback(0, t)
                dfront(1, t)
                dback(1, t)
            dfinal(0)
            dfinal(1)
            nc.sync.dma_start(out=out_d.ap(), in_=yfin)

            dec_loop.close()
    nc.finalize()
    return nc


_prog_cache = {}


def kernel(**inputs):
    from concourse import mybir
    from concourse.bass_utils import run_bass_kernel_spmd

    w, scalars = _prep_weights(inputs)
    fp32 = mybir.dt.float32
    bf16 = mybir.dt.bfloat16
    dt_map = {2: bf16, 4: fp32}
    w_shapes = {
        name: (arr.shape, dt_map[arr.dtype.itemsize]) for name, arr in w.items()
    }

    key = ("v3", tuple(sorted((k, tuple(s), str(d)) for k, (s, d) in w_shapes.items())),
           scalars["wb0"], scalars["c_eff"])
    if key not in _prog_cache:
        _prog_cache[key] = _build(w_shapes, scalars)
    nc = _prog_cache[key]

    full = _f32(inputs["inputs"])
    in_maps = []
    for c in range(NCORES):
        m = {"inp": np.ascontiguousarray(full[c * BL : (c + 1) * BL])}
        m.update(w)
        in_maps.append(m)

    import os

    trace = os.environ.get("DARNN_TRACE", "0") != "0"
    res = run_bass_kernel_spmd(
        nc, in_maps, core_ids=list(range(NCORES)), trace=trace
    )
    global LAST_RESULT
    LAST_RESULT = res
    out = np.concatenate([r["out"] for r in res.results], axis=0)
    return out


LAST_RESULT = None
